# revision 5
# baseline (speedup 1.0000x reference)
"""Trainium2 Bass kernel: custom inverse STFT (degenerate per-bin rotation +
Hann window + overlap-add + window correction).

Math (matching the reference):
    F[i,k]  = S_real[i,k]*A[k] + S_imag[i,k]*B[k]
      A[k]  = w[k]*(cos(th)-sin(th))/n,  B[k] = -w[k]*(cos(th)+sin(th))/n
    out[t]  = sum_i F[i, t-256*i] / max(corr[t], 1e-8)

Sharding: 8192 frames -> 8 cores x 1024 frames.  Core m owns output blocks
[1024m, 1024m+1024) of 256 samples; it loads 3 extra "halo" frames on the
left so every owned block has all 4 overlapping contributions.  The global
tail (blocks 8192..8194, 768 samples) is reconstructed host-side from the
last 3 frames.

On-chip layout per core: frames interleaved as f = 8p + e (partition p gets 8
consecutive frames).  Overlap-add then becomes free-dim-shifted adds; the
per-partition wraparound (blocks whose contributing frame lives on partition
p+1) is handled by DMA-copying each partition's first 3 frames of F down one
partition into a halo tile.
"""

import numpy as np

import concourse.bass as bass
import concourse.bacc as bacc
import concourse.mybir as mybir
import concourse.tile as tile
from concourse.bass_utils import run_bass_kernel_spmd

F32 = mybir.dt.float32
ALU = mybir.AluOpType

P = 128            # SBUF partitions
G = 8              # frames per partition
FL = 1024          # frame length (== fft length)
FS = 256           # frame step
NF = 8192          # total frames
NCORES = 8
FPC = NF // NCORES          # frames owned per core
ROWS = FPC + 3              # input rows per core (3 left-halo frames)
OUT_LEN = FS * (NF - 1) + FL


def _window32():
    # bit-matches the reference's f32 window computation (cancellation in
    # 0.5-0.5*cos makes the f32 rounding of cos visible at the edges, and the
    # output divides by the overlap-added window — numerator and denominator
    # must use the SAME w values for the edge samples to come out right)
    k = np.arange(FL, dtype=np.float32)
    th = np.float32(2.0 * np.pi) * k / np.float32(FL)
    return (np.float32(0.5) - np.float32(0.5) * np.cos(th)).astype(np.float32)


def _coeffs():
    k = np.arange(FL, dtype=np.float64)
    th = 2.0 * np.pi * k / FL
    w = _window32().astype(np.float64)
    a = (w * (np.cos(th) - np.sin(th)) / FL).astype(np.float32)
    b = (-w * (np.cos(th) + np.sin(th)) / FL).astype(np.float32)
    return a, b


def _window_correction():
    w = _window32()
    corr = np.zeros(OUT_LEN, dtype=np.float32)
    # overlap-add of the window; vectorized by chunk-diagonal
    for j in range(4):
        # frame i, chunk j lands at block i+j
        chunk = w[j * FS:(j + 1) * FS]
        blocks = np.arange(NF) + j          # target block index per frame
        view = corr[blocks[0] * FS:(blocks[-1] + 1) * FS].reshape(NF, FS)
        view += chunk[None, :]
    return corr


def build_nc():
    nc = bacc.Bacc(trn_type="TRN2", target_bir_lowering=False, debug=False)
    sr_d = nc.dram_tensor("s_real", [ROWS, FL], F32, kind="ExternalInput").ap()
    si_d = nc.dram_tensor("s_imag", [ROWS, FL], F32, kind="ExternalInput").ap()
    ca_d = nc.dram_tensor("coef_a", [FL], F32, kind="ExternalInput").ap()
    cb_d = nc.dram_tensor("coef_b", [FL], F32, kind="ExternalInput").ap()
    ft_d = nc.dram_tensor("f_tail", [3 * FL], F32, kind="ExternalInput").ap()
    out_d = nc.dram_tensor("out_seg", [FPC * FS], F32, kind="ExternalOutput").ap()

    # [128, 8, 1024] views: partition p holds input rows 8p..8p+7
    sr3 = sr_d[0:P * G, :].rearrange("(p g) k -> p g k", p=P)
    si3 = si_d[0:P * G, :].rearrange("(p g) k -> p g k", p=P)
    out2 = out_d.rearrange("(p x) -> p x", p=P)      # [128, 2048]

    with tile.TileContext(nc) as tc:
        with (
            tc.tile_pool(name="const", bufs=1) as cpool,
            tc.tile_pool(name="main", bufs=1) as mpool,
            tc.tile_pool(name="tmp", bufs=2) as tpool,
        ):
            At = cpool.tile([P, FL], F32, tag="At")
            Bt = cpool.tile([P, FL], F32, tag="Bt")
            Srt = mpool.tile([P, G * FL], F32, tag="Sr")
            Sit = mpool.tile([P, G * FL], F32, tag="Si")
            Ft = mpool.tile([P, G * FL], F32, tag="F")
            Fh = mpool.tile([P, 3 * FL], F32, tag="Fh")
            Ot = mpool.tile([P, G * FS], F32, tag="O")

            # coefficient broadcast across partitions (step-0 DMA)
            nc.sync.dma_start(out=At[:, :], in_=ca_d[None, :].broadcast_to([P, FL]))
            nc.sync.dma_start(out=Bt[:, :], in_=cb_d[None, :].broadcast_to([P, FL]))

            # stream input + elementwise F, one e-slice at a time
            for e in range(G):
                sl = slice(e * FL, (e + 1) * FL)
                nc.sync.dma_start(out=Srt[:, sl], in_=sr3[:, e, :])
                nc.sync.dma_start(out=Sit[:, sl], in_=si3[:, e, :])
                t = tpool.tile([P, FL], F32, tag="t")
                nc.vector.tensor_tensor(out=t[:, :], in0=Sit[:, sl], in1=Bt[:, :], op=ALU.mult)
                nc.vector.tensor_tensor(out=Ft[:, sl], in0=Srt[:, sl], in1=At[:, :], op=ALU.mult)
                nc.vector.tensor_tensor(out=Ft[:, sl], in0=Ft[:, sl], in1=t[:, :], op=ALU.add)

            # halo: partition p gets partition p+1's first 3 frames of F;
            # partition 127 gets the host-computed tail frames.
            nc.sync.dma_start(out=Fh[0:P - 1, :], in_=Ft[1:P, 0:3 * FL])
            nc.sync.dma_start(out=Fh[P - 1:P, :], in_=ft_d[None, :])

            # overlap-add: out[p, b_e] = sum_d F[p, b_e+d, chunk(3-d)]
            Fv = Ft[:, :].rearrange("p (g k) -> p g k", g=G)
            Fhv = Fh[:, :].rearrange("p (g k) -> p g k", g=3)
            Ov = Ot[:, :].rearrange("p (g r) -> p g r", g=G)

            # d = 0 (chunk 3): initializes the output tile
            nc.scalar.copy(out=Ov[:, :, :], in_=Fv[:, :, 3 * FS:4 * FS])
            for d in (1, 2, 3):
                c = 3 - d
                csl = slice(c * FS, (c + 1) * FS)
                nc.vector.tensor_tensor(
                    out=Ov[:, 0:G - d, :], in0=Ov[:, 0:G - d, :],
                    in1=Fv[:, d:G, csl], op=ALU.add)
                nc.vector.tensor_tensor(
                    out=Ov[:, G - d:G, :], in0=Ov[:, G - d:G, :],
                    in1=Fhv[:, 0:d, csl], op=ALU.add)

            nc.sync.dma_start(out=out2, in_=Ot[:, :])
    nc.compile()
    return nc


_cache = {}


def _get_nc():
    if "nc" not in _cache:
        _cache["nc"] = build_nc()
    return _cache["nc"]


def kernel(S_real, S_imag):
    S_real = np.asarray(S_real, dtype=np.float32)
    S_imag = np.asarray(S_imag, dtype=np.float32)
    a, b = _coeffs()

    pad = np.zeros((3, FL), dtype=np.float32)
    sr_pad = np.concatenate([pad, S_real], axis=0)
    si_pad = np.concatenate([pad, S_imag], axis=0)

    in_maps = []
    for m in range(NCORES):
        r0 = m * FPC
        sr_m = np.ascontiguousarray(sr_pad[r0:r0 + ROWS])
        si_m = np.ascontiguousarray(si_pad[r0:r0 + ROWS])
        # host-computed F for this core's last 3 own frames (feeds partition
        # 127's halo slot)
        hi = m * FPC + FPC - 3
        ftl = (S_real[hi:hi + 3] * a[None, :] + S_imag[hi:hi + 3] * b[None, :])
        in_maps.append({
            "s_real": sr_m,
            "s_imag": si_m,
            "coef_a": a,
            "coef_b": b,
            "f_tail": np.ascontiguousarray(ftl.reshape(-1)),
        })

    nc = _get_nc()
    res = run_bass_kernel_spmd(nc, in_maps, list(range(NCORES)))

    out = np.zeros(OUT_LEN, dtype=np.float32)
    for m in range(NCORES):
        seg = res.results[m]["out_seg"]
        out[m * FPC * FS:(m + 1) * FPC * FS] = seg

    # global tail: blocks 8192..8194 from the last 3 frames
    hf = (S_real[NF - 3:] * a[None, :] + S_imag[NF - 3:] * b[None, :])
    for t in range(3):
        i = NF - 3 + t
        for j in range(3 - t, 4):
            blk = i + j
            out[blk * FS:(blk + 1) * FS] += hf[t, j * FS:(j + 1) * FS]

    if "corr" not in _cache:
        _cache["corr"] = _window_correction()
    corr = _cache["corr"]
    return out / np.maximum(corr, np.float32(1e-8))


# revision 7
# speedup vs baseline: 1.5925x; 1.5925x over previous
"""Trainium2 Bass kernel: custom inverse STFT (degenerate per-bin rotation +
Hann window + overlap-add + window correction).

Math (matching the reference):
    F[i,k]  = S_real[i,k]*A[k] + S_imag[i,k]*B[k]
      A[k]  = w[k]*(cos(th)-sin(th))/n,  B[k] = -w[k]*(cos(th)+sin(th))/n
    out[t]  = sum_i F[i, t-256*i] / max(corr[t], 1e-8)

Sharding: 8192 frames -> 8 cores x 1024 frames.  Core m owns output blocks
[1024m, 1024m+1024) of 256 samples; it loads 3 extra "halo" frames on the
left so every owned block has all 4 overlapping contributions.  The global
tail (blocks 8192..8194, 768 samples) is reconstructed host-side from the
last 3 frames.

On-chip layout per core: frames interleaved as f = 8p + e (partition p gets 8
consecutive frames, 32KB contiguous DRAM per partition).  Overlap-add is then
free-dim-shifted adds on the DVE.  The per-partition wraparound (blocks whose
contributing frame lives on partition p+1) is produced by a shift-by-one-
partition matmul on the TensorEngine (lhsT = shifted identity) accumulating
the host-computed tail frames for partition 127 via a one-hot K=1 matmul.

Engine budget per core: DVE does Sr*A and the final add + overlap-add;
GPSIMD does Si*B (runs concurrently - fp32 1x DVE ops use its dedicated SBUF
port pair); ACT does the chunk-3 copies; PE does the halo shift.
"""

import numpy as np

import concourse.bass as bass
import concourse.bacc as bacc
import concourse.mybir as mybir
import concourse.tile as tile
from concourse.bass_utils import run_bass_kernel_spmd

F32 = mybir.dt.float32
ALU = mybir.AluOpType

P = 128            # SBUF partitions
G = 8              # frames per partition
FL = 1024          # frame length (== fft length)
FS = 256           # frame step
NF = 8192          # total frames
NCORES = 8
FPC = NF // NCORES          # frames owned per core
ROWS = FPC + 3              # input rows per core (3 left-halo frames)
OUT_LEN = FS * (NF - 1) + FL


def _window32():
    # bit-matches the reference's f32 window computation (cancellation in
    # 0.5-0.5*cos makes the f32 rounding of cos visible at the edges, and the
    # output divides by the overlap-added window — numerator and denominator
    # must use the SAME w values for the edge samples to come out right)
    k = np.arange(FL, dtype=np.float32)
    th = np.float32(2.0 * np.pi) * k / np.float32(FL)
    return (np.float32(0.5) - np.float32(0.5) * np.cos(th)).astype(np.float32)


def _coeffs():
    k = np.arange(FL, dtype=np.float64)
    th = 2.0 * np.pi * k / FL
    w = _window32().astype(np.float64)
    a = (w * (np.cos(th) - np.sin(th)) / FL).astype(np.float32)
    b = (-w * (np.cos(th) + np.sin(th)) / FL).astype(np.float32)
    return a, b


def _window_correction():
    w = _window32()
    corr = np.zeros(OUT_LEN, dtype=np.float32)
    for j in range(4):
        chunk = w[j * FS:(j + 1) * FS]
        view = corr[j * FS:j * FS + NF * FS].reshape(NF, FS)
        view += chunk[None, :]
    return corr


def _shift_weights():
    # [129, 128]: rows 0..127 = shifted identity (w[p, q] = 1 iff p == q+1),
    # row 128 = one-hot selecting output partition 127 (for the tail K=1
    # accumulation matmul)
    w = np.zeros((P + 1, P), dtype=np.float32)
    w[1:P, np.arange(P - 1)] = 0.0  # placeholder, set below
    for q in range(P - 1):
        w[q + 1, q] = 1.0
    w[P, P - 1] = 1.0
    return w


def build_nc():
    nc = bacc.Bacc(trn_type="TRN2", target_bir_lowering=False, debug=False)
    sr_d = nc.dram_tensor("s_real", [ROWS, FL], F32, kind="ExternalInput").ap()
    si_d = nc.dram_tensor("s_imag", [ROWS, FL], F32, kind="ExternalInput").ap()
    ca_d = nc.dram_tensor("coef_a", [FL], F32, kind="ExternalInput").ap()
    cb_d = nc.dram_tensor("coef_b", [FL], F32, kind="ExternalInput").ap()
    ft_d = nc.dram_tensor("f_tail", [3 * FL], F32, kind="ExternalInput").ap()
    sw_d = nc.dram_tensor("shiftw", [(P + 1) * P], F32, kind="ExternalInput").ap()
    out_d = nc.dram_tensor("out_seg", [FPC * FS], F32, kind="ExternalOutput").ap()

    # [128, 8, 1024] views: partition p holds input rows 8p..8p+7
    sr3 = sr_d[0:P * G, :].rearrange("(p g) k -> p g k", p=P)
    si3 = si_d[0:P * G, :].rearrange("(p g) k -> p g k", p=P)
    sw2 = sw_d.rearrange("(p q) -> p q", p=P + 1)
    out2 = out_d.rearrange("(p x) -> p x", p=P)      # [128, 2048]

    with tile.TileContext(nc) as tc:
        with (
            tc.tile_pool(name="const", bufs=1) as cpool,
            tc.tile_pool(name="main", bufs=1) as mpool,
            tc.tile_pool(name="tmp", bufs=3) as tpool,
            tc.tile_pool(name="psum", bufs=1, space="PSUM") as ppool,
        ):
            At = cpool.tile([P, FL], F32, tag="At")
            Bt = cpool.tile([P, FL], F32, tag="Bt")
            S1 = cpool.tile([P, P], F32, tag="S1")       # shifted identity
            E127 = cpool.tile([1, P], F32, tag="E127")   # one-hot row
            Ttl = cpool.tile([1, 3 * FL], F32, tag="Ttl")
            Srt = mpool.tile([P, G * FL], F32, tag="Sr")
            Sit = mpool.tile([P, G * FL], F32, tag="Si")
            Ft = mpool.tile([P, G * FL], F32, tag="F")
            Ot = mpool.tile([P, G * FS], F32, tag="O")
            Hp = ppool.tile([P, 2048], F32, tag="Hp")    # halo via PE, 4 banks

            # constants: coefficient broadcast (step-0 DMA), shift weights, tail
            nc.sync.dma_start(out=At[:, :], in_=ca_d[None, :].broadcast_to([P, FL]))
            nc.sync.dma_start(out=Bt[:, :], in_=cb_d[None, :].broadcast_to([P, FL]))
            nc.scalar.dma_start(out=S1[:, :], in_=sw2[0:P, :])
            nc.scalar.dma_start(out=E127[:, :], in_=sw2[P:P + 1, :])
            nc.scalar.dma_start(out=Ttl[:, :], in_=ft_d[None, :])

            # stream input + elementwise F, one e-slice at a time
            # DVE: Sr*A and the final add; GPSIMD: Si*B (concurrent)
            for e in range(G):
                sl = slice(e * FL, (e + 1) * FL)
                nc.sync.dma_start(out=Srt[:, sl], in_=sr3[:, e, :])
                nc.sync.dma_start(out=Sit[:, sl], in_=si3[:, e, :])
                t = tpool.tile([P, FL], F32, tag="t")
                nc.gpsimd.tensor_tensor(out=t[:, :], in0=Sit[:, sl], in1=Bt[:, :], op=ALU.mult)
                nc.vector.tensor_tensor(out=Ft[:, sl], in0=Srt[:, sl], in1=At[:, :], op=ALU.mult)
                nc.vector.tensor_tensor(out=Ft[:, sl], in0=Ft[:, sl], in1=t[:, :], op=ALU.add)

            Fv = Ft[:, :].rearrange("p (g k) -> p g k", g=G)
            Tv = Ttl[:, :].rearrange("p (g k) -> p g k", g=3)
            Ov = Ot[:, :].rearrange("p (g r) -> p g r", g=G)

            # halo by PE shift: Hp[q, :] = F[q+1, sel] (+ tail for q=127)
            # layout: [0:768]   = frames 0..2 chunk0   (read by d=3)
            #         [1024:1536] = frames 0..1 chunk1 (read by d=2)
            #         [1536:1792] = frame 0 chunk2     (read by d=1)
            mm = [
                (slice(0, 512),      (slice(0, 2), slice(0, FS))),        # f01 c0
                (slice(512, 768),    (slice(2, 3), slice(0, FS))),        # f2  c0
                (slice(1024, 1536),  (slice(0, 2), slice(FS, 2 * FS))),   # f01 c1
                (slice(1536, 1792),  (slice(0, 1), slice(2 * FS, 3 * FS))),  # f0 c2
            ]
            for osl, (gsl, ksl) in mm:
                nc.tensor.matmul(Hp[:, osl], S1[:, :], Fv[:, gsl, ksl],
                                 start=True, stop=False)
                nc.tensor.matmul(Hp[:, osl], E127[:, :], Tv[:, gsl, ksl],
                                 start=False, stop=True)

            # overlap-add in two halves so half A streams out early.
            # out[p, b_e] = sum_d F[p, b_e+d, chunk(3-d)], wrap terms from Hp
            # half A: b_e 0..3 (needs F e <= 6, no wrap)
            nc.scalar.copy(out=Ov[:, 0:4, :], in_=Fv[:, 0:4, 3 * FS:4 * FS])
            for d in (1, 2, 3):
                c = 3 - d
                csl = slice(c * FS, (c + 1) * FS)
                nc.vector.tensor_tensor(
                    out=Ov[:, 0:4, :], in0=Ov[:, 0:4, :],
                    in1=Fv[:, d:4 + d, csl], op=ALU.add)
            nc.sync.dma_start(out=out2[:, 0:4 * FS], in_=Ot[:, 0:4 * FS])

            # half B: b_e 4..7 (wrap terms read PSUM)
            nc.scalar.copy(out=Ov[:, 4:8, :], in_=Fv[:, 4:8, 3 * FS:4 * FS])
            hp_sl = {1: slice(1536, 1792), 2: slice(1024, 1536), 3: slice(0, 768)}
            for d in (1, 2, 3):
                c = 3 - d
                csl = slice(c * FS, (c + 1) * FS)
                nc.vector.tensor_tensor(
                    out=Ov[:, 4:8 - d, :], in0=Ov[:, 4:8 - d, :],
                    in1=Fv[:, 4 + d:8, csl], op=ALU.add)
                # wrap blocks b_e = 8-d..7 <- Hp (flat slices so shapes match)
                osl = slice((8 - d) * FS, 8 * FS)
                nc.vector.tensor_tensor(
                    out=Ot[:, osl], in0=Ot[:, osl],
                    in1=Hp[:, hp_sl[d]], op=ALU.add)
            nc.scalar.dma_start(out=out2[:, 4 * FS:], in_=Ot[:, 4 * FS:])
    nc.compile()
    return nc


_cache = {}


def _get_nc():
    if "nc" not in _cache:
        _cache["nc"] = build_nc()
    return _cache["nc"]


def make_in_maps(S_real, S_imag):
    a, b = _coeffs()
    pad = np.zeros((3, FL), dtype=np.float32)
    sr_pad = np.concatenate([pad, S_real], axis=0)
    si_pad = np.concatenate([pad, S_imag], axis=0)
    shiftw = _shift_weights().reshape(-1)
    in_maps = []
    for m in range(NCORES):
        r0 = m * FPC
        hi = m * FPC + FPC - 3
        # host-computed F for this core's last 3 own frames (feeds partition
        # 127's halo)
        ftl = (S_real[hi:hi + 3] * a[None, :] + S_imag[hi:hi + 3] * b[None, :])
        in_maps.append({
            "s_real": np.ascontiguousarray(sr_pad[r0:r0 + ROWS]),
            "s_imag": np.ascontiguousarray(si_pad[r0:r0 + ROWS]),
            "coef_a": a,
            "coef_b": b,
            "f_tail": np.ascontiguousarray(ftl.reshape(-1)),
            "shiftw": shiftw,
        })
    return in_maps


def assemble_output(S_real, S_imag, segs):
    a, b = _coeffs()
    out = np.zeros(OUT_LEN, dtype=np.float32)
    for m in range(NCORES):
        out[m * FPC * FS:(m + 1) * FPC * FS] = segs[m]

    # global tail: blocks 8192..8194 from the last 3 frames
    hf = (S_real[NF - 3:] * a[None, :] + S_imag[NF - 3:] * b[None, :])
    for t in range(3):
        i = NF - 3 + t
        for j in range(3 - t, 4):
            blk = i + j
            out[blk * FS:(blk + 1) * FS] += hf[t, j * FS:(j + 1) * FS]

    if "corr" not in _cache:
        _cache["corr"] = _window_correction()
    corr = _cache["corr"]
    return out / np.maximum(corr, np.float32(1e-8))


def kernel(S_real, S_imag):
    S_real = np.asarray(S_real, dtype=np.float32)
    S_imag = np.asarray(S_imag, dtype=np.float32)
    in_maps = make_in_maps(S_real, S_imag)
    nc = _get_nc()
    res = run_bass_kernel_spmd(nc, in_maps, list(range(NCORES)))
    segs = [res.results[m]["out_seg"] for m in range(NCORES)]
    return assemble_output(S_real, S_imag, segs)


# revision 8
# speedup vs baseline: 1.8533x; 1.1638x over previous
"""Trainium2 Bass kernel: custom inverse STFT (degenerate per-bin rotation +
Hann window + overlap-add + window correction).

Math (matching the reference):
    F[i,k]  = S_real[i,k]*A[k] + S_imag[i,k]*B[k]
      A[k]  = w[k]*(cos(th)-sin(th))/n,  B[k] = -w[k]*(cos(th)+sin(th))/n
    out[t]  = sum_i F[i, t-256*i] / max(corr[t], 1e-8)

Sharding: 8192 frames -> 8 cores x 1024 frames.  Core m owns output blocks
[1024m, 1024m+1024) of 256 samples; it loads 3 extra "halo" frames on the
left so every owned block has all 4 overlapping contributions.  The global
tail (blocks 8192..8194, 768 samples) is reconstructed host-side from the
last 3 frames.

On-chip layout per core: frames interleaved as f = 8p + e (partition p gets 8
consecutive frames, 32KB contiguous DRAM per partition).  Overlap-add is then
free-dim-shifted adds on the DVE.  The per-partition wraparound (blocks whose
contributing frame lives on partition p+1) is produced by a shift-by-one-
partition matmul on the TensorEngine (lhsT = shifted identity) accumulating
the host-computed tail frames for partition 127 via a one-hot K=1 matmul.

Engine budget per core: DVE does Sr*A and the final add + overlap-add;
GPSIMD does Si*B (runs concurrently - fp32 1x DVE ops use its dedicated SBUF
port pair); ACT does the chunk-3 copies; PE does the halo shift.
"""

import numpy as np

import concourse.bass as bass
import concourse.bacc as bacc
import concourse.mybir as mybir
import concourse.tile as tile
from concourse.bass_utils import run_bass_kernel_spmd

F32 = mybir.dt.float32
ALU = mybir.AluOpType

P = 128            # SBUF partitions
G = 8              # frames per partition
FL = 1024          # frame length (== fft length)
FS = 256           # frame step
NF = 8192          # total frames
NCORES = 8
FPC = NF // NCORES          # frames owned per core
ROWS = FPC + 3              # input rows per core (3 left-halo frames)
OUT_LEN = FS * (NF - 1) + FL


def _window32():
    # bit-matches the reference's f32 window computation (cancellation in
    # 0.5-0.5*cos makes the f32 rounding of cos visible at the edges, and the
    # output divides by the overlap-added window — numerator and denominator
    # must use the SAME w values for the edge samples to come out right)
    k = np.arange(FL, dtype=np.float32)
    th = np.float32(2.0 * np.pi) * k / np.float32(FL)
    return (np.float32(0.5) - np.float32(0.5) * np.cos(th)).astype(np.float32)


def _coeffs():
    k = np.arange(FL, dtype=np.float64)
    th = 2.0 * np.pi * k / FL
    w = _window32().astype(np.float64)
    a = (w * (np.cos(th) - np.sin(th)) / FL).astype(np.float32)
    b = (-w * (np.cos(th) + np.sin(th)) / FL).astype(np.float32)
    return a, b


def _window_correction():
    w = _window32()
    corr = np.zeros(OUT_LEN, dtype=np.float32)
    for j in range(4):
        chunk = w[j * FS:(j + 1) * FS]
        view = corr[j * FS:j * FS + NF * FS].reshape(NF, FS)
        view += chunk[None, :]
    return corr


def _shift_weights():
    # [129, 128]: rows 0..127 = shifted identity (w[p, q] = 1 iff p == q+1),
    # row 128 = one-hot selecting output partition 127 (for the tail K=1
    # accumulation matmul)
    w = np.zeros((P + 1, P), dtype=np.float32)
    w[1:P, np.arange(P - 1)] = 0.0  # placeholder, set below
    for q in range(P - 1):
        w[q + 1, q] = 1.0
    w[P, P - 1] = 1.0
    return w


def build_nc():
    nc = bacc.Bacc(trn_type="TRN2", target_bir_lowering=False, debug=False)
    sr_d = nc.dram_tensor("s_real", [ROWS, FL], F32, kind="ExternalInput").ap()
    si_d = nc.dram_tensor("s_imag", [ROWS, FL], F32, kind="ExternalInput").ap()
    ca_d = nc.dram_tensor("coef_a", [FL], F32, kind="ExternalInput").ap()
    cb_d = nc.dram_tensor("coef_b", [FL], F32, kind="ExternalInput").ap()
    ft_d = nc.dram_tensor("f_tail", [3 * FL], F32, kind="ExternalInput").ap()
    sw_d = nc.dram_tensor("shiftw", [(P + 1) * P], F32, kind="ExternalInput").ap()
    out_d = nc.dram_tensor("out_seg", [FPC * FS], F32, kind="ExternalOutput").ap()

    # [128, 8, 1024] views: partition p holds input rows 8p..8p+7
    sr3 = sr_d[0:P * G, :].rearrange("(p g) k -> p g k", p=P)
    si3 = si_d[0:P * G, :].rearrange("(p g) k -> p g k", p=P)
    sw2 = sw_d.rearrange("(p q) -> p q", p=P + 1)
    out2 = out_d.rearrange("(p x) -> p x", p=P)      # [128, 2048]

    with tile.TileContext(nc) as tc:
        with (
            tc.tile_pool(name="const", bufs=1) as cpool,
            tc.tile_pool(name="main", bufs=1) as mpool,
            tc.tile_pool(name="tmp", bufs=3) as tpool,
            tc.tile_pool(name="psum", bufs=1, space="PSUM") as ppool,
        ):
            At = cpool.tile([P, FL], F32, tag="At")
            Bt = cpool.tile([P, FL], F32, tag="Bt")
            S1 = cpool.tile([P, P], F32, tag="S1")       # shifted identity
            E127 = cpool.tile([1, P], F32, tag="E127")   # one-hot row
            Ttl = cpool.tile([1, 3 * FL], F32, tag="Ttl")
            Srt = mpool.tile([P, G * FL], F32, tag="Sr")
            Sit = mpool.tile([P, G * FL], F32, tag="Si")
            Ft = mpool.tile([P, G * FL], F32, tag="F")
            Ot = mpool.tile([P, G * FS], F32, tag="O")
            Hp = ppool.tile([P, 2048], F32, tag="Hp")    # halo via PE, 4 banks

            # constants: coefficient broadcast (step-0 DMA), shift weights, tail
            nc.sync.dma_start(out=At[:, :], in_=ca_d[None, :].broadcast_to([P, FL]))
            nc.sync.dma_start(out=Bt[:, :], in_=cb_d[None, :].broadcast_to([P, FL]))
            nc.scalar.dma_start(out=S1[:, :], in_=sw2[0:P, :])
            nc.scalar.dma_start(out=E127[:, :], in_=sw2[P:P + 1, :])
            nc.scalar.dma_start(out=Ttl[:, :], in_=ft_d[None, :])

            # stream input + elementwise F, one e-slice at a time
            # DVE: Sr*A and the final add; GPSIMD: Si*B (concurrent)
            for e in range(G):
                sl = slice(e * FL, (e + 1) * FL)
                nc.sync.dma_start(out=Srt[:, sl], in_=sr3[:, e, :])
                nc.sync.dma_start(out=Sit[:, sl], in_=si3[:, e, :])
                t = tpool.tile([P, FL], F32, tag="t")
                nc.vector.tensor_tensor(out=t[:, :], in0=Sit[:, sl], in1=Bt[:, :], op=ALU.mult)
                nc.vector.tensor_tensor(out=Ft[:, sl], in0=Srt[:, sl], in1=At[:, :], op=ALU.mult)
                nc.vector.tensor_tensor(out=Ft[:, sl], in0=Ft[:, sl], in1=t[:, :], op=ALU.add)

            Fv = Ft[:, :].rearrange("p (g k) -> p g k", g=G)
            Tv = Ttl[:, :].rearrange("p (g k) -> p g k", g=3)
            Ov = Ot[:, :].rearrange("p (g r) -> p g r", g=G)

            # halo by PE shift: Hp[q, :] = F[q+1, sel] (+ tail for q=127)
            # layout: [0:768]   = frames 0..2 chunk0   (read by d=3)
            #         [1024:1536] = frames 0..1 chunk1 (read by d=2)
            #         [1536:1792] = frame 0 chunk2     (read by d=1)
            mm = [
                (slice(0, 512),      (slice(0, 2), slice(0, FS))),        # f01 c0
                (slice(512, 768),    (slice(2, 3), slice(0, FS))),        # f2  c0
                (slice(1024, 1536),  (slice(0, 2), slice(FS, 2 * FS))),   # f01 c1
                (slice(1536, 1792),  (slice(0, 1), slice(2 * FS, 3 * FS))),  # f0 c2
            ]
            for osl, (gsl, ksl) in mm:
                nc.tensor.matmul(Hp[:, osl], S1[:, :], Fv[:, gsl, ksl],
                                 start=True, stop=False)
                nc.tensor.matmul(Hp[:, osl], E127[:, :], Tv[:, gsl, ksl],
                                 start=False, stop=True)

            # overlap-add in two halves so half A streams out early.
            # out[p, b_e] = sum_d F[p, b_e+d, chunk(3-d)], wrap terms from Hp
            # half A: b_e 0..3 (needs F e <= 6, no wrap)
            nc.scalar.copy(out=Ov[:, 0:4, :], in_=Fv[:, 0:4, 3 * FS:4 * FS])
            for d in (1, 2, 3):
                c = 3 - d
                csl = slice(c * FS, (c + 1) * FS)
                nc.vector.tensor_tensor(
                    out=Ov[:, 0:4, :], in0=Ov[:, 0:4, :],
                    in1=Fv[:, d:4 + d, csl], op=ALU.add)
            nc.sync.dma_start(out=out2[:, 0:4 * FS], in_=Ot[:, 0:4 * FS])

            # half B: b_e 4..7 (wrap terms read PSUM)
            nc.scalar.copy(out=Ov[:, 4:8, :], in_=Fv[:, 4:8, 3 * FS:4 * FS])
            hp_sl = {1: slice(1536, 1792), 2: slice(1024, 1536), 3: slice(0, 768)}
            for d in (1, 2, 3):
                c = 3 - d
                csl = slice(c * FS, (c + 1) * FS)
                nc.vector.tensor_tensor(
                    out=Ov[:, 4:8 - d, :], in0=Ov[:, 4:8 - d, :],
                    in1=Fv[:, 4 + d:8, csl], op=ALU.add)
                # wrap blocks b_e = 8-d..7 <- Hp (flat slices so shapes match)
                osl = slice((8 - d) * FS, 8 * FS)
                nc.vector.tensor_tensor(
                    out=Ot[:, osl], in0=Ot[:, osl],
                    in1=Hp[:, hp_sl[d]], op=ALU.add)
            nc.scalar.dma_start(out=out2[:, 4 * FS:], in_=Ot[:, 4 * FS:])
    nc.compile()
    return nc


_cache = {}


def _get_nc():
    if "nc" not in _cache:
        _cache["nc"] = build_nc()
    return _cache["nc"]


def make_in_maps(S_real, S_imag):
    a, b = _coeffs()
    pad = np.zeros((3, FL), dtype=np.float32)
    sr_pad = np.concatenate([pad, S_real], axis=0)
    si_pad = np.concatenate([pad, S_imag], axis=0)
    shiftw = _shift_weights().reshape(-1)
    in_maps = []
    for m in range(NCORES):
        r0 = m * FPC
        hi = m * FPC + FPC - 3
        # host-computed F for this core's last 3 own frames (feeds partition
        # 127's halo)
        ftl = (S_real[hi:hi + 3] * a[None, :] + S_imag[hi:hi + 3] * b[None, :])
        in_maps.append({
            "s_real": np.ascontiguousarray(sr_pad[r0:r0 + ROWS]),
            "s_imag": np.ascontiguousarray(si_pad[r0:r0 + ROWS]),
            "coef_a": a,
            "coef_b": b,
            "f_tail": np.ascontiguousarray(ftl.reshape(-1)),
            "shiftw": shiftw,
        })
    return in_maps


def assemble_output(S_real, S_imag, segs):
    a, b = _coeffs()
    out = np.zeros(OUT_LEN, dtype=np.float32)
    for m in range(NCORES):
        out[m * FPC * FS:(m + 1) * FPC * FS] = segs[m]

    # global tail: blocks 8192..8194 from the last 3 frames
    hf = (S_real[NF - 3:] * a[None, :] + S_imag[NF - 3:] * b[None, :])
    for t in range(3):
        i = NF - 3 + t
        for j in range(3 - t, 4):
            blk = i + j
            out[blk * FS:(blk + 1) * FS] += hf[t, j * FS:(j + 1) * FS]

    if "corr" not in _cache:
        _cache["corr"] = _window_correction()
    corr = _cache["corr"]
    return out / np.maximum(corr, np.float32(1e-8))


def kernel(S_real, S_imag):
    S_real = np.asarray(S_real, dtype=np.float32)
    S_imag = np.asarray(S_imag, dtype=np.float32)
    in_maps = make_in_maps(S_real, S_imag)
    nc = _get_nc()
    res = run_bass_kernel_spmd(nc, in_maps, list(range(NCORES)))
    segs = [res.results[m]["out_seg"] for m in range(NCORES)]
    return assemble_output(S_real, S_imag, segs)


# revision 10
# speedup vs baseline: 1.8641x; 1.0058x over previous
"""Trainium2 Bass kernel: custom inverse STFT (degenerate per-bin rotation +
Hann window + overlap-add + window correction).

Math (matching the reference):
    F[i,k]  = S_real[i,k]*A[k] + S_imag[i,k]*B[k]
      A[k]  = w[k]*(cos(th)-sin(th))/n,  B[k] = -w[k]*(cos(th)+sin(th))/n
    out[t]  = sum_i F[i, t-256*i] / max(corr[t], 1e-8)

Sharding: 8192 frames -> 8 cores x 1024 frames.  Core m owns output blocks
[1024m, 1024m+1024) of 256 samples; it loads 3 extra "halo" frames on the
left so every owned block has all 4 overlapping contributions.  The global
tail (blocks 8192..8194, 768 samples) is reconstructed host-side from the
last 3 frames.

On-chip layout per core: frames interleaved as f = 8p + e (partition p gets 8
consecutive frames, 32KB contiguous DRAM per partition).  Overlap-add is then
free-dim-shifted adds on the DVE.  The per-partition wraparound (blocks whose
contributing frame lives on partition p+1) is produced by a shift-by-one-
partition matmul on the TensorEngine (lhsT = shifted identity) accumulating
the host-computed tail frames for partition 127 via a one-hot K=1 matmul.

Engine budget per core: DVE does Sr*A and the final add + overlap-add;
GPSIMD does Si*B (runs concurrently - fp32 1x DVE ops use its dedicated SBUF
port pair); ACT does the chunk-3 copies; PE does the halo shift.
"""

import numpy as np

import concourse.bass as bass
import concourse.bacc as bacc
import concourse.mybir as mybir
import concourse.tile as tile
from concourse.bass_utils import run_bass_kernel_spmd

F32 = mybir.dt.float32
ALU = mybir.AluOpType

P = 128            # SBUF partitions
G = 8              # frames per partition
FL = 1024          # frame length (== fft length)
FS = 256           # frame step
NF = 8192          # total frames
NCORES = 8
FPC = NF // NCORES          # frames owned per core
ROWS = FPC + 3              # input rows per core (3 left-halo frames)
OUT_LEN = FS * (NF - 1) + FL


def _window32():
    # bit-matches the reference's f32 window computation (cancellation in
    # 0.5-0.5*cos makes the f32 rounding of cos visible at the edges, and the
    # output divides by the overlap-added window — numerator and denominator
    # must use the SAME w values for the edge samples to come out right)
    k = np.arange(FL, dtype=np.float32)
    th = np.float32(2.0 * np.pi) * k / np.float32(FL)
    return (np.float32(0.5) - np.float32(0.5) * np.cos(th)).astype(np.float32)


def _coeffs():
    k = np.arange(FL, dtype=np.float64)
    th = 2.0 * np.pi * k / FL
    w = _window32().astype(np.float64)
    a = (w * (np.cos(th) - np.sin(th)) / FL).astype(np.float32)
    b = (-w * (np.cos(th) + np.sin(th)) / FL).astype(np.float32)
    return a, b


def _window_correction():
    w = _window32()
    corr = np.zeros(OUT_LEN, dtype=np.float32)
    for j in range(4):
        chunk = w[j * FS:(j + 1) * FS]
        view = corr[j * FS:j * FS + NF * FS].reshape(NF, FS)
        view += chunk[None, :]
    return corr


def _shift_weights():
    # [129, 128]: rows 0..127 = shifted identity (w[p, q] = 1 iff p == q+1),
    # row 128 = one-hot selecting output partition 127 (for the tail K=1
    # accumulation matmul)
    w = np.zeros((P + 1, P), dtype=np.float32)
    w[1:P, np.arange(P - 1)] = 0.0  # placeholder, set below
    for q in range(P - 1):
        w[q + 1, q] = 1.0
    w[P, P - 1] = 1.0
    return w


def build_nc():
    nc = bacc.Bacc(trn_type="TRN2", target_bir_lowering=False, debug=False)
    sr_d = nc.dram_tensor("s_real", [ROWS, FL], F32, kind="ExternalInput").ap()
    si_d = nc.dram_tensor("s_imag", [ROWS, FL], F32, kind="ExternalInput").ap()
    ca_d = nc.dram_tensor("coef_a", [FL], F32, kind="ExternalInput").ap()
    cb_d = nc.dram_tensor("coef_b", [FL], F32, kind="ExternalInput").ap()
    ft_d = nc.dram_tensor("f_tail", [3 * FL], F32, kind="ExternalInput").ap()
    sw_d = nc.dram_tensor("shiftw", [(P + 1) * P], F32, kind="ExternalInput").ap()
    out_d = nc.dram_tensor("out_seg", [FPC * FS], F32, kind="ExternalOutput").ap()

    # [128, 8, 1024] views: partition p holds input rows 8p..8p+7
    sr3 = sr_d[0:P * G, :].rearrange("(p g) k -> p g k", p=P)
    si3 = si_d[0:P * G, :].rearrange("(p g) k -> p g k", p=P)
    sw2 = sw_d.rearrange("(p q) -> p q", p=P + 1)
    out2 = out_d.rearrange("(p x) -> p x", p=P)      # [128, 2048]

    with tile.TileContext(nc) as tc:
        with (
            tc.tile_pool(name="const", bufs=1) as cpool,
            tc.tile_pool(name="main", bufs=1) as mpool,
            tc.tile_pool(name="tmp", bufs=3) as tpool,
            tc.tile_pool(name="psum", bufs=1, space="PSUM") as ppool,
        ):
            At = cpool.tile([P, FL], F32, tag="At")
            Bt = cpool.tile([P, FL], F32, tag="Bt")
            S1 = cpool.tile([P, P], F32, tag="S1")       # shifted identity
            E127 = cpool.tile([1, P], F32, tag="E127")   # one-hot row
            Ttl = cpool.tile([1, 3 * FL], F32, tag="Ttl")
            Srt = mpool.tile([P, G * FL], F32, tag="Sr")
            Sit = mpool.tile([P, G * FL], F32, tag="Si")
            Ft = mpool.tile([P, G * FL], F32, tag="F")
            Ot = mpool.tile([P, G * FS], F32, tag="O")
            Hp = ppool.tile([P, 2048], F32, tag="Hp")    # halo via PE, 4 banks

            # constants: coefficient broadcast (step-0 DMA) on the tensor
            # engine's queue so it doesn't delay the input stream on sync
            nc.scalar.dma_start(out=At[:, :], in_=ca_d[None, :].broadcast_to([P, FL]))
            nc.scalar.dma_start(out=Bt[:, :], in_=cb_d[None, :].broadcast_to([P, FL]))
            nc.scalar.dma_start(out=S1[:, :], in_=sw2[0:P, :])
            nc.scalar.dma_start(out=E127[:, :], in_=sw2[P:P + 1, :])
            nc.scalar.dma_start(out=Ttl[:, :], in_=ft_d[None, :])

            # stream input + elementwise F, one e-slice at a time
            # DVE: Sr*A and the final add; GPSIMD: Si*B (concurrent)
            for e in range(G):
                sl = slice(e * FL, (e + 1) * FL)
                nc.sync.dma_start(out=Srt[:, sl], in_=sr3[:, e, :])
                nc.sync.dma_start(out=Sit[:, sl], in_=si3[:, e, :])
                t = tpool.tile([P, FL], F32, tag="t")
                nc.vector.tensor_tensor(out=t[:, :], in0=Sit[:, sl], in1=Bt[:, :], op=ALU.mult)
                nc.vector.tensor_tensor(out=Ft[:, sl], in0=Srt[:, sl], in1=At[:, :], op=ALU.mult)
                nc.vector.tensor_tensor(out=Ft[:, sl], in0=Ft[:, sl], in1=t[:, :], op=ALU.add)

            Fv = Ft[:, :].rearrange("p (g k) -> p g k", g=G)
            Tv = Ttl[:, :].rearrange("p (g k) -> p g k", g=3)
            Ov = Ot[:, :].rearrange("p (g r) -> p g r", g=G)

            # halo by PE shift: Hp[q, :] = F[q+1, sel] (+ tail for q=127)
            # layout: [0:768]   = frames 0..2 chunk0   (read by d=3)
            #         [1024:1536] = frames 0..1 chunk1 (read by d=2)
            #         [1536:1792] = frame 0 chunk2     (read by d=1)
            mm = [
                (slice(0, 512),      (slice(0, 2), slice(0, FS))),        # f01 c0
                (slice(512, 768),    (slice(2, 3), slice(0, FS))),        # f2  c0
                (slice(1024, 1536),  (slice(0, 2), slice(FS, 2 * FS))),   # f01 c1
                (slice(1536, 1792),  (slice(0, 1), slice(2 * FS, 3 * FS))),  # f0 c2
            ]
            for osl, (gsl, ksl) in mm:
                nc.tensor.matmul(Hp[:, osl], S1[:, :], Fv[:, gsl, ksl],
                                 start=True, stop=False)
                nc.tensor.matmul(Hp[:, osl], E127[:, :], Tv[:, gsl, ksl],
                                 start=False, stop=True)

            # overlap-add in two halves so half A streams out early.
            # out[p, b_e] = sum_d F[p, b_e+d, chunk(3-d)], wrap terms from Hp
            # half A: b_e 0..3 (needs F e <= 6, no wrap)
            nc.scalar.copy(out=Ov[:, 0:4, :], in_=Fv[:, 0:4, 3 * FS:4 * FS])
            for d in (1, 2, 3):
                c = 3 - d
                csl = slice(c * FS, (c + 1) * FS)
                nc.vector.tensor_tensor(
                    out=Ov[:, 0:4, :], in0=Ov[:, 0:4, :],
                    in1=Fv[:, d:4 + d, csl], op=ALU.add)
            nc.sync.dma_start(out=out2[:, 0:4 * FS], in_=Ot[:, 0:4 * FS])

            # half B: b_e 4..7 (wrap terms read PSUM)
            nc.scalar.copy(out=Ov[:, 4:8, :], in_=Fv[:, 4:8, 3 * FS:4 * FS])
            hp_sl = {1: slice(1536, 1792), 2: slice(1024, 1536), 3: slice(0, 768)}
            for d in (1, 2, 3):
                c = 3 - d
                csl = slice(c * FS, (c + 1) * FS)
                nc.vector.tensor_tensor(
                    out=Ov[:, 4:8 - d, :], in0=Ov[:, 4:8 - d, :],
                    in1=Fv[:, 4 + d:8, csl], op=ALU.add)
                # wrap blocks b_e = 8-d..7 <- Hp (flat slices so shapes match)
                osl = slice((8 - d) * FS, 8 * FS)
                nc.vector.tensor_tensor(
                    out=Ot[:, osl], in0=Ot[:, osl],
                    in1=Hp[:, hp_sl[d]], op=ALU.add)
            nc.scalar.dma_start(out=out2[:, 4 * FS:], in_=Ot[:, 4 * FS:])
    nc.compile()
    return nc


_cache = {}


def _get_nc():
    if "nc" not in _cache:
        _cache["nc"] = build_nc()
    return _cache["nc"]


def make_in_maps(S_real, S_imag):
    a, b = _coeffs()
    pad = np.zeros((3, FL), dtype=np.float32)
    sr_pad = np.concatenate([pad, S_real], axis=0)
    si_pad = np.concatenate([pad, S_imag], axis=0)
    shiftw = _shift_weights().reshape(-1)
    in_maps = []
    for m in range(NCORES):
        r0 = m * FPC
        hi = m * FPC + FPC - 3
        # host-computed F for this core's last 3 own frames (feeds partition
        # 127's halo)
        ftl = (S_real[hi:hi + 3] * a[None, :] + S_imag[hi:hi + 3] * b[None, :])
        in_maps.append({
            "s_real": np.ascontiguousarray(sr_pad[r0:r0 + ROWS]),
            "s_imag": np.ascontiguousarray(si_pad[r0:r0 + ROWS]),
            "coef_a": a,
            "coef_b": b,
            "f_tail": np.ascontiguousarray(ftl.reshape(-1)),
            "shiftw": shiftw,
        })
    return in_maps


def assemble_output(S_real, S_imag, segs):
    a, b = _coeffs()
    out = np.zeros(OUT_LEN, dtype=np.float32)
    for m in range(NCORES):
        out[m * FPC * FS:(m + 1) * FPC * FS] = segs[m]

    # global tail: blocks 8192..8194 from the last 3 frames
    hf = (S_real[NF - 3:] * a[None, :] + S_imag[NF - 3:] * b[None, :])
    for t in range(3):
        i = NF - 3 + t
        for j in range(3 - t, 4):
            blk = i + j
            out[blk * FS:(blk + 1) * FS] += hf[t, j * FS:(j + 1) * FS]

    if "corr" not in _cache:
        _cache["corr"] = _window_correction()
    corr = _cache["corr"]
    return out / np.maximum(corr, np.float32(1e-8))


def kernel(S_real, S_imag):
    S_real = np.asarray(S_real, dtype=np.float32)
    S_imag = np.asarray(S_imag, dtype=np.float32)
    in_maps = make_in_maps(S_real, S_imag)
    nc = _get_nc()
    res = run_bass_kernel_spmd(nc, in_maps, list(range(NCORES)))
    segs = [res.results[m]["out_seg"] for m in range(NCORES)]
    return assemble_output(S_real, S_imag, segs)
